# revision 34
# baseline (speedup 1.0000x reference)
"""CAN per-sample 2-layer MLP kernel for Trainium2 (8 NeuronCores, SPMD).

Computation (per sample b):
    x = user_emb[b]                           # (50, 16)
    W0, b0, W1, b1 unpacked from item_emb[b]  # (16,16),(16,),(16,16),(16,)
    y = relu(relu(x @ W0 + b0) @ W1 + b1)     # (50, 16)

Mapping:
  * Pure data parallel over 8 cores (2048 samples each).
  * Host packs x^T per sample with an appended ones-row (homogeneous
    coordinates); bias is folded into a 17x17 Wt0 = [[W0,0],[b0,1]] and a
    17x16 Wt1 = [[W1],[b1]] so `x_t @ Wt` applies bias, and the ones row
    self-propagates through layer 1 (relu(1)=1).
  * On chip: 4 samples share a 128-partition tile at 32-row strides so each
    sample's K=17 matmul runs at its own PE tile_position (32j, 32j); the
    four matmuls execute concurrently in distinct array quadrants.
  * x and W interleave in ONE combined DRAM tensor, batch-major so every
    DMA has a contiguous DRAM side. DMA batches are large (G quads) to
    amortize the ~2us per-lane completion latency; PSUM works in GS-quad
    sub-batches (one bank per tile). Input DMAs are issued 2 batches ahead
    on an explicitly chained SP queue so loads overlap compute.
  * Walrus codegen caps inline sync waits (DMACopy/Matmult: 1): a post-pass
    moves excess waits onto NoOps inserted before the instruction on the
    same queue. SBUF-side DMA APs must stay 2D ([row, nparts], [1, run]) -
    the DIRECT2D DMA struct cannot roll a free run across partitions.
"""

from contextlib import ExitStack

import numpy as np

import concourse.bass as bass
import concourse.mybir as mybir
from concourse import tile
from concourse.bass_utils import run_bass_kernel_spmd
from concourse.tile_rust import add_dep_helper

# Problem constants (hardcoded per contract)
B, N, D = 16384, 50, 16
NCORES = 8
BC = B // NCORES            # 2048 samples per core
K = D + 1                   # 17 rows: 16 features + homogeneous ones row
WC = K + D                  # 33 weight cols: 17 (layer0 incl ones col) + 16 (layer1)
CC = N + WC                 # 83 combined cols per quad-slot: x (50) then w (33)
QUADS = BC // 4             # 512 quads of 4 samples per core
G = 32                      # quads per DMA batch
GS = 8                      # quads per PSUM sub-batch (one bank)

F32 = mybir.dt.float32


def _strip_covered_waits(nc):
    """Remove, from DMACopy instructions, semaphore waits already guaranteed
    by an earlier instruction on the same engine queue. Coverage is killed
    for a sem from the point of any non-increment update (barrier resets)."""
    for fn in nc.m.functions:
        for blk in fn.blocks:
            seen = {}
            for ins in blk.instructions:
                si = ins.sync_info
                if si is None:
                    continue
                eng = ins.engine
                strippable = type(ins).__name__ == "InstDMACopy"
                kept = []
                changed = False
                for w in si.on_wait:
                    if (
                        strippable
                        and w.wait_mode == "sem-ge-imm"
                        and w.wait_reg is None
                        and seen.get((eng, w.id), -1) >= w.wait_value
                    ):
                        changed = True
                        continue
                    kept.append(w)
                for w in kept:
                    if w.wait_mode == "sem-ge-imm" and w.wait_reg is None:
                        key = (eng, w.id)
                        if seen.get(key, -1) < w.wait_value:
                            seen[key] = w.wait_value
                for u in si.on_update:
                    if u.update_mode != "sem-add-imm" or (
                        u.update_value is not None and u.update_value < 0
                    ):
                        for key in [k for k in seen if k[1] == u.id]:
                            del seen[key]
                if changed:
                    ins.sync_info = mybir.SyncInfo(
                        on_wait=kept, on_update=si.on_update
                    )


_WS_COUNT = [0]


def _split_excess_waits(nc, cap=1):
    """Move excess inline waits onto NoOps inserted immediately before, on
    the same engine queue - semantically identical (sequencers execute
    waits in order)."""
    for fn in nc.m.functions:
        for blk in fn.blocks:
            insts = blk.instructions
            i = 0
            while i < len(insts):
                ins = insts[i]
                si = ins.sync_info
                if si is None or len(si.on_wait) <= cap:
                    i += 1
                    continue
                waits = list(si.on_wait)
                keep, extra = waits[-cap:], waits[:-cap]
                ins.sync_info = mybir.SyncInfo(on_wait=keep, on_update=si.on_update)
                for w in extra:
                    _WS_COUNT[0] += 1
                    nop = mybir.InstNoOp(name=f"I-ws{_WS_COUNT[0]}", ins=[], outs=[])
                    nop.engine = ins.engine
                    nop.sync_info = mybir.SyncInfo(on_wait=[w], on_update=[])
                    insts.insert(i, nop)
                    i += 1
                i += 1


def build_nc(nq=QUADS, g=G, gs=GS, dt=F32, sim_mode=False):
    """Build the per-core Bass program.

    DRAM (per core), batch-major:
      ch [nbatch, 4*K, g*CC] : row K*j+d, col qq*CC+c = sample (bi*g+qq)*4+j
      yh [nbatch, 4*D, g*N]  : row D*j+e, col qq*N+n  = y[n, e] of same
    """
    assert nq % g == 0 and g % gs == 0
    nbatch = nq // g
    nsub = g // gs
    cf = CC * g              # ct data cols per batch
    xf = N * g               # yt data cols per batch
    sf = N * gs              # psum/ht cols per sub-batch
    cfp = cf + 8             # padded row widths: keep SBUF DMA APs 2D
    xfp = xf + 8

    nc = bass.Bass(
        "TRN2",
        target_bir_lowering=False,
        debug=False,
        detect_race_conditions=False,  # post-pass NoOps confuse its bookkeeping
    )
    ch = nc.dram_tensor("ch", [nbatch, 4 * K, cf], dt, kind="ExternalInput")
    yh = nc.dram_tensor("yh", [nbatch, 4 * D, xf], F32, kind="ExternalOutput")

    relu = mybir.ActivationFunctionType.Relu

    with tile.TileContext(nc) as tc, ExitStack() as ctx:
        cpool = ctx.enter_context(tc.tile_pool(name="cpool", bufs=4))
        hpool = ctx.enter_context(tc.tile_pool(name="hpool", bufs=3))
        ypool = ctx.enter_context(tc.tile_pool(name="ypool", bufs=3))
        pspool = ctx.enter_context(tc.tile_pool(name="ps", bufs=4, space="PSUM"))

        prev_sp = [None]

        def sp_chain(inst):
            # Pin SP issue order to emission order so prefetched loads are
            # dispatched before later batches' stores.
            if prev_sp[0] is not None:
                add_dep_helper(inst.ins, prev_sp[0].ins, sync=False,
                               reason="SP issue order")
            prev_sp[0] = inst
            return inst

        cts = {}

        def emit_in_dma(bi):
            ct = cpool.tile([128, cfp], dt, name="ct")
            cts[bi] = ct
            for j in range(4):
                # SWDGE (gpsimd): HWDGE assigns DRAM-sourced loads to a
                # single SDMA engine (observed: one engine 98% busy while
                # 15 idle); SWDGE sprays descriptors across engines by
                # destination partition.
                nc.gpsimd.dma_start(
                    bass.AP(ct.tensor, 32 * j * cfp, [[cfp, K], [1, cf]]),
                    bass.AP(ch, (bi * 4 * K + j * K) * cf, [[cf, K], [1, cf]]),
                )

        # 3-deep prefetch prologue
        for pb in range(min(3, nbatch)):
            emit_in_dma(pb)

        # Software-pipelined emission with one-sub-batch skew: the PE queue
        # is in-order, so L2(s) (which waits on relu1(s)) is emitted AFTER
        # L1(s+1) - the PE computes layer 1 of the next sub-batch while
        # relu1(s) runs on ACT, instead of stalling.
        subs = [(bi, s) for bi in range(nbatch) for s in range(nsub)]
        state = {}   # ss -> (ct, yt, ps1, ht)
        yts = {}

        def emit_l1(ss):
            bi, s = subs[ss]
            if s == 0:
                if bi + 3 < nbatch:
                    emit_in_dma(bi + 3)
                yts[bi] = ypool.tile([128, xfp], F32, name="yt")
            ct = cts[bi]
            ps1 = pspool.tile([128, sf], F32, name="ps1")
            if sim_mode:
                nc.vector.memset(ps1[:, :], 0.0)
            for q in range(gs):
                qq = s * gs + q
                for j in range(4):
                    nc.tensor.matmul(
                        bass.AP(ps1.tensor, 32 * j * sf + q * N, [[sf, K], [1, N]]),
                        bass.AP(ct.tensor, 32 * j * cfp + qq * CC + N, [[cfp, K], [1, K]]),
                        bass.AP(ct.tensor, 32 * j * cfp + qq * CC, [[cfp, K], [1, N]]),
                        start=True,
                        stop=True,
                        tile_position=(32 * j, 32 * j),
                    )
            ht = hpool.tile([128, sf], dt, name="ht")
            nc.scalar.activation(ht[:, :], ps1[:, :], relu)
            state[ss] = (ct, ht)

        def emit_l2(ss):
            bi, s = subs[ss]
            ct, ht = state.pop(ss)
            yt = yts[bi]
            ps2 = pspool.tile([128, sf], F32, name="ps2")
            if sim_mode:
                nc.vector.memset(ps2[:, :], 0.0)
            for q in range(gs):
                qq = s * gs + q
                for j in range(4):
                    nc.tensor.matmul(
                        bass.AP(ps2.tensor, 32 * j * sf + q * N, [[sf, D], [1, N]]),
                        bass.AP(ct.tensor, 32 * j * cfp + qq * CC + N + K, [[cfp, K], [1, D]]),
                        bass.AP(ht.tensor, 32 * j * sf + q * N, [[sf, K], [1, N]]),
                        start=True,
                        stop=True,
                        tile_position=(32 * j, 32 * j),
                    )
            # relu2 writes this sub-batch's slice of yt (DVE)
            nc.vector.tensor_scalar_max(
                bass.AP(yt.tensor, s * sf, [[xfp, 128], [1, sf]]),
                ps2[:, :],
                0.0,
            )
            if s == nsub - 1:
                for j in range(4):
                    sp_chain(nc.sync.dma_start(
                        bass.AP(yh, (bi * 4 * D + j * D) * xf, [[xf, D], [1, xf]]),
                        bass.AP(yt.tensor, 32 * j * xfp, [[xfp, D], [1, xf]]),
                    ))
                cts.pop(bi)

        # skew 2: L2(s) is emitted after L1(s+2) so two sub-batches of
        # layer-1 matmuls cover the relu1 chain latency on the in-order PE queue
        SKEW = 2
        for idx in range(len(subs) + SKEW):
            if idx < len(subs):
                emit_l1(idx)
            if idx >= SKEW:
                emit_l2(idx - SKEW)

    _strip_covered_waits(nc)
    _split_excess_waits(nc)
    return nc


def pack_inputs(user_emb, item_emb, nq=QUADS, g=G, dt=np.float32):
    """Shard + lay out inputs for the 8 cores. Returns list of in_maps."""
    ncores = NCORES
    nbatch = nq // g
    x = np.ascontiguousarray(user_emb, dtype=np.float32)
    ie = np.ascontiguousarray(item_emb, dtype=np.float32)
    btot = ncores * nq * 4

    comb = np.empty((btot, K, CC), dtype=np.float32)
    comb[:, :D, :N] = x[:btot].transpose(0, 2, 1)
    comb[:, D, :N] = 1.0
    w = comb[:, :, N:]
    w[:, :D, :D] = ie[:btot, : D * D].reshape(btot, D, D)          # W0
    w[:, D, :D] = ie[:btot, D * D : D * D + D]                     # b0
    w[:, :D, D] = 0.0
    w[:, D, D] = 1.0                                               # ones col
    off = D * (D + 1)
    w[:, :D, K : K + D] = ie[:btot, off : off + D * D].reshape(btot, D, D)  # W1
    w[:, D, K : K + D] = ie[:btot, off + D * D : off + D * D + D]  # b1

    chs = (
        comb.reshape(ncores, nbatch, g, 4, K, CC)
        .transpose(0, 1, 3, 4, 2, 5)       # c, bi, j, d, qq, col
        .astype(dt, copy=False)
    )
    return [
        {"ch": np.ascontiguousarray(chs[c]).reshape(nbatch, 4 * K, g * CC)}
        for c in range(ncores)
    ]


def unpack_output(results, nq=QUADS, g=G):
    """results: per-core {"yh": [nbatch, 4*D, g*N]} -> full (B, N, D) f32."""
    nbatch = nq // g
    yh = np.stack([r["yh"] for r in results])
    y = (
        yh.reshape(NCORES, nbatch, 4, D, g, N)
        .transpose(0, 1, 4, 2, 5, 3)            # c, bi, qq, j, n, e
    )
    return np.ascontiguousarray(y.reshape(NCORES * nq * 4, N, D))


_NC_CACHE = {}


def _get_nc(key=(QUADS, G)):
    if key not in _NC_CACHE:
        nq, g = key
        _NC_CACHE[key] = build_nc(nq=nq, g=g)
    return _NC_CACHE[key]


def kernel(user_emb, item_emb):
    nc = _get_nc()
    in_maps = pack_inputs(user_emb, item_emb)
    res = run_bass_kernel_spmd(nc, in_maps, core_ids=list(range(NCORES)))
    return unpack_output(res.results)


# revision 35
# speedup vs baseline: 1.0825x; 1.0825x over previous
"""CAN per-sample 2-layer MLP kernel for Trainium2 (8 NeuronCores, SPMD).

Computation (per sample b):
    x = user_emb[b]                           # (50, 16)
    W0, b0, W1, b1 unpacked from item_emb[b]  # (16,16),(16,),(16,16),(16,)
    y = relu(relu(x @ W0 + b0) @ W1 + b1)     # (50, 16)

Mapping:
  * Pure data parallel over 8 cores (2048 samples each).
  * Host packs x^T per sample with an appended ones-row (homogeneous
    coordinates); bias is folded into a 17x17 Wt0 = [[W0,0],[b0,1]] and a
    17x16 Wt1 = [[W1],[b1]] so `x_t @ Wt` applies bias, and the ones row
    self-propagates through layer 1 (relu(1)=1).
  * On chip: 4 samples share a 128-partition tile at 32-row strides so each
    sample's K=17 matmul runs at its own PE tile_position (32j, 32j); the
    four matmuls execute concurrently in distinct array quadrants.
  * x and W interleave in ONE combined DRAM tensor, batch-major so every
    DMA has a contiguous DRAM side. DMA batches are large (G quads) to
    amortize the ~2us per-lane completion latency; PSUM works in GS-quad
    sub-batches (one bank per tile). Input DMAs are issued 2 batches ahead
    on an explicitly chained SP queue so loads overlap compute.
  * Walrus codegen caps inline sync waits (DMACopy/Matmult: 1): a post-pass
    moves excess waits onto NoOps inserted before the instruction on the
    same queue. SBUF-side DMA APs must stay 2D ([row, nparts], [1, run]) -
    the DIRECT2D DMA struct cannot roll a free run across partitions.
"""

from contextlib import ExitStack

import numpy as np

import concourse.bass as bass
import concourse.mybir as mybir
from concourse import tile
from concourse.bass_utils import run_bass_kernel_spmd
from concourse.tile_rust import add_dep_helper

# Problem constants (hardcoded per contract)
B, N, D = 16384, 50, 16
NCORES = 8
BC = B // NCORES            # 2048 samples per core
K = D + 1                   # 17 rows: 16 features + homogeneous ones row
WC = K + D                  # 33 weight cols: 17 (layer0 incl ones col) + 16 (layer1)
CC = N + WC                 # 83 combined cols per quad-slot: x (50) then w (33)
QUADS = BC // 4             # 512 quads of 4 samples per core
G = 32                      # quads per DMA batch
GS = 8                      # quads per PSUM sub-batch (one bank)

F32 = mybir.dt.float32


def _strip_covered_waits(nc):
    """Remove, from DMACopy instructions, semaphore waits already guaranteed
    by an earlier instruction on the same engine queue. Coverage is killed
    for a sem from the point of any non-increment update (barrier resets)."""
    for fn in nc.m.functions:
        for blk in fn.blocks:
            seen = {}
            for ins in blk.instructions:
                si = ins.sync_info
                if si is None:
                    continue
                eng = ins.engine
                strippable = type(ins).__name__ == "InstDMACopy"
                kept = []
                changed = False
                for w in si.on_wait:
                    if (
                        strippable
                        and w.wait_mode == "sem-ge-imm"
                        and w.wait_reg is None
                        and seen.get((eng, w.id), -1) >= w.wait_value
                    ):
                        changed = True
                        continue
                    kept.append(w)
                for w in kept:
                    if w.wait_mode == "sem-ge-imm" and w.wait_reg is None:
                        key = (eng, w.id)
                        if seen.get(key, -1) < w.wait_value:
                            seen[key] = w.wait_value
                for u in si.on_update:
                    if u.update_mode != "sem-add-imm" or (
                        u.update_value is not None and u.update_value < 0
                    ):
                        for key in [k for k in seen if k[1] == u.id]:
                            del seen[key]
                if changed:
                    ins.sync_info = mybir.SyncInfo(
                        on_wait=kept, on_update=si.on_update
                    )


_WS_COUNT = [0]


def _split_excess_waits(nc, cap=1):
    """Move excess inline waits onto NoOps inserted immediately before, on
    the same engine queue - semantically identical (sequencers execute
    waits in order)."""
    for fn in nc.m.functions:
        for blk in fn.blocks:
            insts = blk.instructions
            i = 0
            while i < len(insts):
                ins = insts[i]
                si = ins.sync_info
                if si is None or len(si.on_wait) <= cap:
                    i += 1
                    continue
                waits = list(si.on_wait)
                keep, extra = waits[-cap:], waits[:-cap]
                ins.sync_info = mybir.SyncInfo(on_wait=keep, on_update=si.on_update)
                for w in extra:
                    _WS_COUNT[0] += 1
                    nop = mybir.InstNoOp(name=f"I-ws{_WS_COUNT[0]}", ins=[], outs=[])
                    nop.engine = ins.engine
                    nop.sync_info = mybir.SyncInfo(on_wait=[w], on_update=[])
                    insts.insert(i, nop)
                    i += 1
                i += 1


def build_nc(nq=QUADS, g=G, gs=GS, dt=F32, sim_mode=False):
    """Build the per-core Bass program.

    DRAM (per core), batch-major:
      ch [nbatch, 4*K, g*CC] : row K*j+d, col qq*CC+c = sample (bi*g+qq)*4+j
      yh [nbatch, 4*D, g*N]  : row D*j+e, col qq*N+n  = y[n, e] of same
    """
    assert nq % g == 0 and g % gs == 0
    nbatch = nq // g
    nsub = g // gs
    cf = CC * g              # ct data cols per batch
    xf = N * g               # yt data cols per batch
    sf = N * gs              # psum/ht cols per sub-batch
    cfp = cf + 8             # padded row widths: keep SBUF DMA APs 2D
    xfp = xf + 8

    nc = bass.Bass(
        "TRN2",
        target_bir_lowering=False,
        debug=False,
        detect_race_conditions=False,  # post-pass NoOps confuse its bookkeeping
    )
    ch = nc.dram_tensor("ch", [nbatch, 4 * K, cf], dt, kind="ExternalInput")
    yh = nc.dram_tensor("yh", [nbatch, 4 * D, xf], F32, kind="ExternalOutput")

    relu = mybir.ActivationFunctionType.Relu

    with tile.TileContext(nc) as tc, ExitStack() as ctx:
        cpool = ctx.enter_context(tc.tile_pool(name="cpool", bufs=3))
        hpool = ctx.enter_context(tc.tile_pool(name="hpool", bufs=3))
        ypool = ctx.enter_context(tc.tile_pool(name="ypool", bufs=2))
        pspool = ctx.enter_context(tc.tile_pool(name="ps", bufs=4, space="PSUM"))

        prev_sp = [None]

        def sp_chain(inst):
            # Pin SP issue order to emission order so prefetched loads are
            # dispatched before later batches' stores.
            if prev_sp[0] is not None:
                add_dep_helper(inst.ins, prev_sp[0].ins, sync=False,
                               reason="SP issue order")
            prev_sp[0] = inst
            return inst

        cts = {}

        def emit_in_dma(bi):
            ct = cpool.tile([128, cfp], dt, name="ct")
            cts[bi] = ct
            for j in range(4):
                # SWDGE (gpsimd): HWDGE assigns DRAM-sourced loads to a
                # single SDMA engine (observed: one engine 98% busy while
                # 15 idle); SWDGE sprays descriptors across engines by
                # destination partition.
                nc.gpsimd.dma_start(
                    bass.AP(ct.tensor, 32 * j * cfp, [[cfp, K], [1, cf]]),
                    bass.AP(ch, (bi * 4 * K + j * K) * cf, [[cf, K], [1, cf]]),
                )

        # 2-deep prefetch prologue
        for pb in range(min(2, nbatch)):
            emit_in_dma(pb)

        # Software-pipelined emission with one-sub-batch skew: the PE queue
        # is in-order, so L2(s) (which waits on relu1(s)) is emitted AFTER
        # L1(s+1) - the PE computes layer 1 of the next sub-batch while
        # relu1(s) runs on ACT, instead of stalling.
        subs = [(bi, s) for bi in range(nbatch) for s in range(nsub)]
        state = {}   # ss -> (ct, yt, ps1, ht)
        yts = {}

        def emit_l1(ss):
            bi, s = subs[ss]
            if s == 0:
                if bi + 2 < nbatch:
                    emit_in_dma(bi + 2)
                yts[bi] = ypool.tile([128, xfp], F32, name="yt")
            ct = cts[bi]
            ps1 = pspool.tile([128, sf], F32, name="ps1")
            if sim_mode:
                nc.vector.memset(ps1[:, :], 0.0)
            for q in range(gs):
                qq = s * gs + q
                for j in range(4):
                    nc.tensor.matmul(
                        bass.AP(ps1.tensor, 32 * j * sf + q * N, [[sf, K], [1, N]]),
                        bass.AP(ct.tensor, 32 * j * cfp + qq * CC + N, [[cfp, K], [1, K]]),
                        bass.AP(ct.tensor, 32 * j * cfp + qq * CC, [[cfp, K], [1, N]]),
                        start=True,
                        stop=True,
                        tile_position=(32 * j, 32 * j),
                    )
            ht = hpool.tile([128, sf], dt, name="ht")
            nc.scalar.activation(ht[:, :], ps1[:, :], relu)
            state[ss] = (ct, ht)

        def emit_l2(ss):
            bi, s = subs[ss]
            ct, ht = state.pop(ss)
            yt = yts[bi]
            ps2 = pspool.tile([128, sf], F32, name="ps2")
            if sim_mode:
                nc.vector.memset(ps2[:, :], 0.0)
            for q in range(gs):
                qq = s * gs + q
                for j in range(4):
                    nc.tensor.matmul(
                        bass.AP(ps2.tensor, 32 * j * sf + q * N, [[sf, D], [1, N]]),
                        bass.AP(ct.tensor, 32 * j * cfp + qq * CC + N + K, [[cfp, K], [1, D]]),
                        bass.AP(ht.tensor, 32 * j * sf + q * N, [[sf, K], [1, N]]),
                        start=True,
                        stop=True,
                        tile_position=(32 * j, 32 * j),
                    )
            # relu2 writes this sub-batch's slice of yt (DVE)
            nc.vector.tensor_scalar_max(
                bass.AP(yt.tensor, s * sf, [[xfp, 128], [1, sf]]),
                ps2[:, :],
                0.0,
            )
            if s == nsub - 1:
                for j in range(4):
                    sp_chain(nc.sync.dma_start(
                        bass.AP(yh, (bi * 4 * D + j * D) * xf, [[xf, D], [1, xf]]),
                        bass.AP(yt.tensor, 32 * j * xfp, [[xfp, D], [1, xf]]),
                    ))
                cts.pop(bi)

        # skew 2: L2(s) is emitted after L1(s+2) so two sub-batches of
        # layer-1 matmuls cover the relu1 chain latency on the in-order PE queue
        SKEW = 1
        for idx in range(len(subs) + SKEW):
            if idx < len(subs):
                emit_l1(idx)
            if idx >= SKEW:
                emit_l2(idx - SKEW)

    _strip_covered_waits(nc)
    _split_excess_waits(nc)
    return nc


def pack_inputs(user_emb, item_emb, nq=QUADS, g=G, dt=np.float32):
    """Shard + lay out inputs for the 8 cores. Returns list of in_maps."""
    ncores = NCORES
    nbatch = nq // g
    x = np.ascontiguousarray(user_emb, dtype=np.float32)
    ie = np.ascontiguousarray(item_emb, dtype=np.float32)
    btot = ncores * nq * 4

    comb = np.empty((btot, K, CC), dtype=np.float32)
    comb[:, :D, :N] = x[:btot].transpose(0, 2, 1)
    comb[:, D, :N] = 1.0
    w = comb[:, :, N:]
    w[:, :D, :D] = ie[:btot, : D * D].reshape(btot, D, D)          # W0
    w[:, D, :D] = ie[:btot, D * D : D * D + D]                     # b0
    w[:, :D, D] = 0.0
    w[:, D, D] = 1.0                                               # ones col
    off = D * (D + 1)
    w[:, :D, K : K + D] = ie[:btot, off : off + D * D].reshape(btot, D, D)  # W1
    w[:, D, K : K + D] = ie[:btot, off + D * D : off + D * D + D]  # b1

    chs = (
        comb.reshape(ncores, nbatch, g, 4, K, CC)
        .transpose(0, 1, 3, 4, 2, 5)       # c, bi, j, d, qq, col
        .astype(dt, copy=False)
    )
    return [
        {"ch": np.ascontiguousarray(chs[c]).reshape(nbatch, 4 * K, g * CC)}
        for c in range(ncores)
    ]


def unpack_output(results, nq=QUADS, g=G):
    """results: per-core {"yh": [nbatch, 4*D, g*N]} -> full (B, N, D) f32."""
    nbatch = nq // g
    yh = np.stack([r["yh"] for r in results])
    y = (
        yh.reshape(NCORES, nbatch, 4, D, g, N)
        .transpose(0, 1, 4, 2, 5, 3)            # c, bi, qq, j, n, e
    )
    return np.ascontiguousarray(y.reshape(NCORES * nq * 4, N, D))


_NC_CACHE = {}


def _get_nc(key=(QUADS, G)):
    if key not in _NC_CACHE:
        nq, g = key
        _NC_CACHE[key] = build_nc(nq=nq, g=g)
    return _NC_CACHE[key]


def kernel(user_emb, item_emb):
    nc = _get_nc()
    in_maps = pack_inputs(user_emb, item_emb)
    res = run_bass_kernel_spmd(nc, in_maps, core_ids=list(range(NCORES)))
    return unpack_output(res.results)


# revision 36
# speedup vs baseline: 1.1109x; 1.0263x over previous
"""CAN per-sample 2-layer MLP kernel for Trainium2 (8 NeuronCores, SPMD).

Computation (per sample b):
    x = user_emb[b]                           # (50, 16)
    W0, b0, W1, b1 unpacked from item_emb[b]  # (16,16),(16,),(16,16),(16,)
    y = relu(relu(x @ W0 + b0) @ W1 + b1)     # (50, 16)

Mapping:
  * Pure data parallel over 8 cores (2048 samples each).
  * Host packs x^T per sample with an appended ones-row (homogeneous
    coordinates); bias is folded into a 17x17 Wt0 = [[W0,0],[b0,1]] and a
    17x16 Wt1 = [[W1],[b1]] so `x_t @ Wt` applies bias, and the ones row
    self-propagates through layer 1 (relu(1)=1).
  * On chip: 4 samples share a 128-partition tile at 32-row strides so each
    sample's K=17 matmul runs at its own PE tile_position (32j, 32j); the
    four matmuls execute concurrently in distinct array quadrants.
  * x and W interleave in ONE combined DRAM tensor, batch-major so every
    DMA has a contiguous DRAM side. DMA batches are large (G quads) to
    amortize the ~2us per-lane completion latency; PSUM works in GS-quad
    sub-batches (one bank per tile). Input DMAs are issued 2 batches ahead
    on an explicitly chained SP queue so loads overlap compute.
  * Walrus codegen caps inline sync waits (DMACopy/Matmult: 1): a post-pass
    moves excess waits onto NoOps inserted before the instruction on the
    same queue. SBUF-side DMA APs must stay 2D ([row, nparts], [1, run]) -
    the DIRECT2D DMA struct cannot roll a free run across partitions.
"""

from contextlib import ExitStack

import numpy as np

import concourse.bass as bass
import concourse.mybir as mybir
from concourse import tile
from concourse.bass_utils import run_bass_kernel_spmd
from concourse.tile_rust import add_dep_helper

# Problem constants (hardcoded per contract)
B, N, D = 16384, 50, 16
NCORES = 8
BC = B // NCORES            # 2048 samples per core
K = D + 1                   # 17 rows: 16 features + homogeneous ones row
WC = K + D                  # 33 weight cols: 17 (layer0 incl ones col) + 16 (layer1)
CC = N + WC                 # 83 combined cols per quad-slot: x (50) then w (33)
QUADS = BC // 4             # 512 quads of 4 samples per core
G = 32                      # quads per DMA batch
GS = 8                      # quads per PSUM sub-batch (one bank)

F32 = mybir.dt.float32


def _strip_covered_waits(nc):
    """Remove, from DMACopy instructions, semaphore waits already guaranteed
    by an earlier instruction on the same engine queue. Coverage is killed
    for a sem from the point of any non-increment update (barrier resets)."""
    for fn in nc.m.functions:
        for blk in fn.blocks:
            seen = {}
            for ins in blk.instructions:
                si = ins.sync_info
                if si is None:
                    continue
                eng = ins.engine
                strippable = type(ins).__name__ == "InstDMACopy"
                kept = []
                changed = False
                for w in si.on_wait:
                    if (
                        strippable
                        and w.wait_mode == "sem-ge-imm"
                        and w.wait_reg is None
                        and seen.get((eng, w.id), -1) >= w.wait_value
                    ):
                        changed = True
                        continue
                    kept.append(w)
                for w in kept:
                    if w.wait_mode == "sem-ge-imm" and w.wait_reg is None:
                        key = (eng, w.id)
                        if seen.get(key, -1) < w.wait_value:
                            seen[key] = w.wait_value
                for u in si.on_update:
                    if u.update_mode != "sem-add-imm" or (
                        u.update_value is not None and u.update_value < 0
                    ):
                        for key in [k for k in seen if k[1] == u.id]:
                            del seen[key]
                if changed:
                    ins.sync_info = mybir.SyncInfo(
                        on_wait=kept, on_update=si.on_update
                    )


_WS_COUNT = [0]


def _split_excess_waits(nc, cap=1):
    """Move excess inline waits onto NoOps inserted immediately before, on
    the same engine queue - semantically identical (sequencers execute
    waits in order)."""
    for fn in nc.m.functions:
        for blk in fn.blocks:
            insts = blk.instructions
            i = 0
            while i < len(insts):
                ins = insts[i]
                si = ins.sync_info
                if si is None or len(si.on_wait) <= cap:
                    i += 1
                    continue
                waits = list(si.on_wait)
                keep, extra = waits[-cap:], waits[:-cap]
                ins.sync_info = mybir.SyncInfo(on_wait=keep, on_update=si.on_update)
                for w in extra:
                    _WS_COUNT[0] += 1
                    nop = mybir.InstNoOp(name=f"I-ws{_WS_COUNT[0]}", ins=[], outs=[])
                    nop.engine = ins.engine
                    nop.sync_info = mybir.SyncInfo(on_wait=[w], on_update=[])
                    insts.insert(i, nop)
                    i += 1
                i += 1


def build_nc(nq=QUADS, g=G, gs=GS, dt=F32, sim_mode=False):
    """Build the per-core Bass program.

    DRAM (per core), batch-major:
      ch [nbatch, 4*K, g*CC] : row K*j+d, col qq*CC+c = sample (bi*g+qq)*4+j
      yh [nbatch, 4*D, g*N]  : row D*j+e, col qq*N+n  = y[n, e] of same
    """
    assert nq % g == 0 and g % gs == 0
    nbatch = nq // g
    nsub = g // gs
    cf = CC * g              # ct data cols per batch
    xf = N * g               # yt data cols per batch
    sf = N * gs              # psum/ht cols per sub-batch
    cfp = cf + 8             # padded row widths: keep SBUF DMA APs 2D
    xfp = xf + 8

    nc = bass.Bass(
        "TRN2",
        target_bir_lowering=False,
        debug=False,
        detect_race_conditions=False,  # post-pass NoOps confuse its bookkeeping
    )
    ch = nc.dram_tensor("ch", [nbatch, 4 * K, cf], dt, kind="ExternalInput")
    yh = nc.dram_tensor("yh", [nbatch, 4 * D, xf], F32, kind="ExternalOutput")

    relu = mybir.ActivationFunctionType.Relu

    with tile.TileContext(nc) as tc, ExitStack() as ctx:
        cpool = ctx.enter_context(tc.tile_pool(name="cpool", bufs=3))
        hpool = ctx.enter_context(tc.tile_pool(name="hpool", bufs=3))
        ypool = ctx.enter_context(tc.tile_pool(name="ypool", bufs=2))
        pspool = ctx.enter_context(tc.tile_pool(name="ps", bufs=4, space="PSUM"))

        prev_sp = [None]

        def sp_chain(inst):
            # Pin SP issue order to emission order so prefetched loads are
            # dispatched before later batches' stores.
            if prev_sp[0] is not None:
                add_dep_helper(inst.ins, prev_sp[0].ins, sync=False,
                               reason="SP issue order")
            prev_sp[0] = inst
            return inst

        cts = {}

        def emit_in_dma(bi):
            ct = cpool.tile([128, cfp], dt, name="ct")
            cts[bi] = ct
            for j in range(4):
                # SWDGE (gpsimd): HWDGE assigns DRAM-sourced loads to a
                # single SDMA engine (observed: one engine 98% busy while
                # 15 idle); SWDGE sprays descriptors across engines by
                # destination partition.
                nc.gpsimd.dma_start(
                    bass.AP(ct.tensor, 32 * j * cfp, [[cfp, K], [1, cf]]),
                    bass.AP(ch, (bi * 4 * K + j * K) * cf, [[cf, K], [1, cf]]),
                )

        # 2-deep prefetch prologue
        for pb in range(min(2, nbatch)):
            emit_in_dma(pb)

        # Software-pipelined emission with one-sub-batch skew: the PE queue
        # is in-order, so L2(s) (which waits on relu1(s)) is emitted AFTER
        # L1(s+1) - the PE computes layer 1 of the next sub-batch while
        # relu1(s) runs on ACT, instead of stalling.
        subs = [(bi, s) for bi in range(nbatch) for s in range(nsub)]
        state = {}   # ss -> (ct, yt, ps1, ht)
        yts = {}

        def emit_l1(ss):
            bi, s = subs[ss]
            if s == 0:
                if bi + 2 < nbatch:
                    emit_in_dma(bi + 2)
                yts[bi] = ypool.tile([128, xfp], F32, name="yt")
            ct = cts[bi]
            ps1 = pspool.tile([128, sf], F32, name="ps1")
            if sim_mode:
                nc.vector.memset(ps1[:, :], 0.0)
            for q in range(gs):
                qq = s * gs + q
                for j in range(4):
                    nc.tensor.matmul(
                        bass.AP(ps1.tensor, 32 * j * sf + q * N, [[sf, K], [1, N]]),
                        bass.AP(ct.tensor, 32 * j * cfp + qq * CC + N, [[cfp, K], [1, K]]),
                        bass.AP(ct.tensor, 32 * j * cfp + qq * CC, [[cfp, K], [1, N]]),
                        start=True,
                        stop=True,
                        tile_position=(32 * j, 32 * j),
                    )
            ht = hpool.tile([128, sf], dt, name="ht")
            # split relu1 across ACT+DVE halves: halves the latency on the
            # layer-2 critical path; subtile deps let each half's L2 matmuls
            # start independently
            h2 = sf // 2
            nc.scalar.activation(ht[:, :h2], ps1[:, :h2], relu)
            nc.vector.tensor_scalar_max(ht[:, h2:], ps1[:, h2:], 0.0)
            state[ss] = (ct, ht)

        def emit_l2(ss):
            bi, s = subs[ss]
            ct, ht = state.pop(ss)
            yt = yts[bi]
            ps2 = pspool.tile([128, sf], F32, name="ps2")
            if sim_mode:
                nc.vector.memset(ps2[:, :], 0.0)
            for q in range(gs):
                qq = s * gs + q
                for j in range(4):
                    nc.tensor.matmul(
                        bass.AP(ps2.tensor, 32 * j * sf + q * N, [[sf, D], [1, N]]),
                        bass.AP(ct.tensor, 32 * j * cfp + qq * CC + N + K, [[cfp, K], [1, D]]),
                        bass.AP(ht.tensor, 32 * j * sf + q * N, [[sf, K], [1, N]]),
                        start=True,
                        stop=True,
                        tile_position=(32 * j, 32 * j),
                    )
            # relu2 split across DVE+ACT halves
            h2 = sf // 2
            nc.vector.tensor_scalar_max(
                bass.AP(yt.tensor, s * sf, [[xfp, 128], [1, h2]]),
                ps2[:, :h2],
                0.0,
            )
            nc.scalar.activation(
                bass.AP(yt.tensor, s * sf + h2, [[xfp, 128], [1, sf - h2]]),
                ps2[:, h2:],
                relu,
            )
            if s == nsub - 1:
                for j in range(4):
                    sp_chain(nc.sync.dma_start(
                        bass.AP(yh, (bi * 4 * D + j * D) * xf, [[xf, D], [1, xf]]),
                        bass.AP(yt.tensor, 32 * j * xfp, [[xfp, D], [1, xf]]),
                    ))
                cts.pop(bi)

        # skew 2: L2(s) is emitted after L1(s+2) so two sub-batches of
        # layer-1 matmuls cover the relu1 chain latency on the in-order PE queue
        SKEW = 1
        for idx in range(len(subs) + SKEW):
            if idx < len(subs):
                emit_l1(idx)
            if idx >= SKEW:
                emit_l2(idx - SKEW)

    _strip_covered_waits(nc)
    _split_excess_waits(nc)
    return nc


def pack_inputs(user_emb, item_emb, nq=QUADS, g=G, dt=np.float32):
    """Shard + lay out inputs for the 8 cores. Returns list of in_maps."""
    ncores = NCORES
    nbatch = nq // g
    x = np.ascontiguousarray(user_emb, dtype=np.float32)
    ie = np.ascontiguousarray(item_emb, dtype=np.float32)
    btot = ncores * nq * 4

    comb = np.empty((btot, K, CC), dtype=np.float32)
    comb[:, :D, :N] = x[:btot].transpose(0, 2, 1)
    comb[:, D, :N] = 1.0
    w = comb[:, :, N:]
    w[:, :D, :D] = ie[:btot, : D * D].reshape(btot, D, D)          # W0
    w[:, D, :D] = ie[:btot, D * D : D * D + D]                     # b0
    w[:, :D, D] = 0.0
    w[:, D, D] = 1.0                                               # ones col
    off = D * (D + 1)
    w[:, :D, K : K + D] = ie[:btot, off : off + D * D].reshape(btot, D, D)  # W1
    w[:, D, K : K + D] = ie[:btot, off + D * D : off + D * D + D]  # b1

    chs = (
        comb.reshape(ncores, nbatch, g, 4, K, CC)
        .transpose(0, 1, 3, 4, 2, 5)       # c, bi, j, d, qq, col
        .astype(dt, copy=False)
    )
    return [
        {"ch": np.ascontiguousarray(chs[c]).reshape(nbatch, 4 * K, g * CC)}
        for c in range(ncores)
    ]


def unpack_output(results, nq=QUADS, g=G):
    """results: per-core {"yh": [nbatch, 4*D, g*N]} -> full (B, N, D) f32."""
    nbatch = nq // g
    yh = np.stack([r["yh"] for r in results])
    y = (
        yh.reshape(NCORES, nbatch, 4, D, g, N)
        .transpose(0, 1, 4, 2, 5, 3)            # c, bi, qq, j, n, e
    )
    return np.ascontiguousarray(y.reshape(NCORES * nq * 4, N, D))


_NC_CACHE = {}


def _get_nc(key=(QUADS, G)):
    if key not in _NC_CACHE:
        nq, g = key
        _NC_CACHE[key] = build_nc(nq=nq, g=g)
    return _NC_CACHE[key]


def kernel(user_emb, item_emb):
    nc = _get_nc()
    in_maps = pack_inputs(user_emb, item_emb)
    res = run_bass_kernel_spmd(nc, in_maps, core_ids=list(range(NCORES)))
    return unpack_output(res.results)


# revision 39
# speedup vs baseline: 1.4851x; 1.3368x over previous
"""CAN per-sample 2-layer MLP kernel for Trainium2 (8 NeuronCores, SPMD).

Computation (per sample b):
    x = user_emb[b]                           # (50, 16)
    W0, b0, W1, b1 unpacked from item_emb[b]  # (16,16),(16,),(16,16),(16,)
    y = relu(relu(x @ W0 + b0) @ W1 + b1)     # (50, 16)

Mapping:
  * Pure data parallel over 8 cores (2048 samples each).
  * Host packs x^T per sample with an appended ones-row (homogeneous
    coordinates); bias is folded into a 17x17 Wt0 = [[W0,0],[b0,1]] and a
    17x16 Wt1 = [[W1],[b1]] so `x_t @ Wt` applies bias, and the ones row
    self-propagates through layer 1 (relu(1)=1).
  * On chip: 4 samples share a 128-partition tile at 32-row strides so each
    sample's K=17 matmul runs at its own PE tile_position (32j, 32j); the
    four matmuls execute concurrently in distinct array quadrants.
  * x and W interleave in ONE combined DRAM tensor, batch-major so every
    DMA has a contiguous DRAM side. DMA batches are large (G quads) to
    amortize the ~2us per-lane completion latency; PSUM works in GS-quad
    sub-batches (one bank per tile). Input DMAs are issued 2 batches ahead
    via SWDGE (HWDGE pins DRAM-sourced loads to one SDMA engine) and the
    PE stream is software-pipelined with a one-sub-batch skew; relus are
    split across the ACT and DVE engines.
  * Walrus codegen caps inline sync waits (DMACopy/Matmult: 1): a post-pass
    moves excess waits onto NoOps inserted before the instruction on the
    same queue. SBUF-side DMA APs must stay 2D ([row, nparts], [1, run]) -
    the DIRECT2D DMA struct cannot roll a free run across partitions.
"""

from contextlib import ExitStack

import numpy as np

import concourse.bass as bass
import concourse.mybir as mybir
from concourse import tile
from concourse.bass_utils import run_bass_kernel_spmd
from concourse.tile_rust import add_dep_helper

# Problem constants (hardcoded per contract)
B, N, D = 16384, 50, 16
NCORES = 8
BC = B // NCORES            # 2048 samples per core
K = D + 1                   # 17 rows per sample: 16 features + ones row
KP = 2 * K                  # 34 rows per sample-pair
WP = KP + 2 * D             # 66 weight cols per pair: 34 (L1 blockdiag) + 32 (L2)
CC = N + WP                 # 116 combined cols per pair-slot: x (50) then w (66)
QUADS = BC // 4             # 512 quads of 4 samples per core
G = 32                      # quads per DMA batch
GS = 8                      # quads per PSUM sub-batch (one bank)

F32 = mybir.dt.float32


def _strip_covered_waits(nc):
    """Remove, from DMACopy instructions, semaphore waits already guaranteed
    by an earlier instruction on the same engine queue. Coverage is killed
    for a sem from the point of any non-increment update (barrier resets)."""
    for fn in nc.m.functions:
        for blk in fn.blocks:
            seen = {}
            for ins in blk.instructions:
                si = ins.sync_info
                if si is None:
                    continue
                eng = ins.engine
                strippable = type(ins).__name__ == "InstDMACopy"
                kept = []
                changed = False
                for w in si.on_wait:
                    if (
                        strippable
                        and w.wait_mode == "sem-ge-imm"
                        and w.wait_reg is None
                        and seen.get((eng, w.id), -1) >= w.wait_value
                    ):
                        changed = True
                        continue
                    kept.append(w)
                for w in kept:
                    if w.wait_mode == "sem-ge-imm" and w.wait_reg is None:
                        key = (eng, w.id)
                        if seen.get(key, -1) < w.wait_value:
                            seen[key] = w.wait_value
                for u in si.on_update:
                    if u.update_mode != "sem-add-imm" or (
                        u.update_value is not None and u.update_value < 0
                    ):
                        for key in [k for k in seen if k[1] == u.id]:
                            del seen[key]
                if changed:
                    ins.sync_info = mybir.SyncInfo(
                        on_wait=kept, on_update=si.on_update
                    )


_WS_COUNT = [0]


def _split_excess_waits(nc, cap=1):
    """Move excess inline waits onto NoOps inserted immediately before, on
    the same engine queue - semantically identical (sequencers execute
    waits in order)."""
    for fn in nc.m.functions:
        for blk in fn.blocks:
            insts = blk.instructions
            i = 0
            while i < len(insts):
                ins = insts[i]
                si = ins.sync_info
                if si is None or len(si.on_wait) <= cap:
                    i += 1
                    continue
                waits = list(si.on_wait)
                keep, extra = waits[-cap:], waits[:-cap]
                ins.sync_info = mybir.SyncInfo(on_wait=keep, on_update=si.on_update)
                for w in extra:
                    _WS_COUNT[0] += 1
                    nop = mybir.InstNoOp(name=f"I-ws{_WS_COUNT[0]}", ins=[], outs=[])
                    nop.engine = ins.engine
                    nop.sync_info = mybir.SyncInfo(on_wait=[w], on_update=[])
                    insts.insert(i, nop)
                    i += 1
                i += 1


def build_nc(nq=QUADS, g=G, gs=GS, dt=F32, sim_mode=False):
    """Build the per-core Bass program.

    DRAM (per core), batch-major:
      ch [nbatch, 2*KP, g*CC] : row KP*j2+r, col qq*CC+c = pair (bi*g+qq)*2+j2
      yh [nbatch, 2*2*D, g*N] : row 2*D*j2+e2, col qq*N+n = pair outputs
    """
    assert nq % g == 0 and g % gs == 0
    nbatch = nq // g
    nsub = g // gs
    cf = CC * g              # ct data cols per batch
    xf = N * g               # yt data cols per batch
    sf = N * gs              # psum/ht cols per sub-batch
    cfp = cf + 8             # padded row widths: keep SBUF DMA APs 2D
    xfp = xf + 8

    nc = bass.Bass(
        "TRN2",
        target_bir_lowering=False,
        debug=False,
        detect_race_conditions=False,  # post-pass NoOps confuse its bookkeeping
    )
    ch = nc.dram_tensor("ch", [nbatch, 2 * KP, cf], dt, kind="ExternalInput")
    yh = nc.dram_tensor("yh", [nbatch, 4 * D, xf], F32, kind="ExternalOutput")

    relu = mybir.ActivationFunctionType.Relu

    with tile.TileContext(nc) as tc, ExitStack() as ctx:
        cpool = ctx.enter_context(tc.tile_pool(name="cpool", bufs=3))
        hpool = ctx.enter_context(tc.tile_pool(name="hpool", bufs=3))
        ypool = ctx.enter_context(tc.tile_pool(name="ypool", bufs=2))
        pspool = ctx.enter_context(tc.tile_pool(name="ps", bufs=4, space="PSUM"))

        prev_sp = [None]

        def sp_chain(inst):
            # Pin SP issue order to emission order so prefetched loads are
            # dispatched before later batches' stores.
            if prev_sp[0] is not None:
                add_dep_helper(inst.ins, prev_sp[0].ins, sync=False,
                               reason="SP issue order")
            prev_sp[0] = inst
            return inst

        cts = {}

        def emit_in_dma(bi):
            ct = cpool.tile([128, cfp], dt, name="ct")
            cts[bi] = ct
            for j2 in range(2):
                # SWDGE: HWDGE pins DRAM-sourced loads to one SDMA engine;
                # SWDGE sprays descriptors across engines by dest partition.
                nc.gpsimd.dma_start(
                    bass.AP(ct.tensor, 64 * j2 * cfp, [[cfp, KP], [1, cf]]),
                    bass.AP(ch, (bi * 2 + j2) * KP * cf, [[cf, KP], [1, cf]]),
                )

        # 2-deep prefetch prologue
        for pb in range(min(2, nbatch)):
            emit_in_dma(pb)

        # Software-pipelined emission with one-sub-batch skew: the PE queue
        # is in-order, so L2(s) (which waits on relu1(s)) is emitted AFTER
        # L1(s+1) - the PE computes layer 1 of the next sub-batch while
        # relu1(s) runs on ACT, instead of stalling.
        subs = [(bi, s) for bi in range(nbatch) for s in range(nsub)]
        state = {}   # ss -> (ct, yt, ps1, ht)
        yts = {}

        def emit_l1(ss):
            bi, s = subs[ss]
            if s == 0:
                if bi + 2 < nbatch:
                    emit_in_dma(bi + 2)
                yts[bi] = ypool.tile([128, xfp], F32, name="yt")
            ct = cts[bi]
            ps1 = pspool.tile([128, sf], F32, name="ps1")
            if sim_mode:
                nc.vector.memset(ps1[:, :], 0.0)
            for q in range(gs):
                qq = s * gs + q
                for j2 in range(2):
                    nc.tensor.matmul(
                        bass.AP(ps1.tensor, 64 * j2 * sf + q * N, [[sf, KP], [1, N]]),
                        bass.AP(ct.tensor, 64 * j2 * cfp + qq * CC + N, [[cfp, KP], [1, KP]]),
                        bass.AP(ct.tensor, 64 * j2 * cfp + qq * CC, [[cfp, KP], [1, N]]),
                        start=True,
                        stop=True,
                        tile_position=(64 * j2, 64 * j2),
                    )
            ht = hpool.tile([128, sf], dt, name="ht")
            # split relu1 across ACT+DVE halves: halves the latency on the
            # layer-2 critical path; subtile deps let each half's L2 matmuls
            # start independently
            h2 = sf // 2
            nc.scalar.activation(ht[:, :h2], ps1[:, :h2], relu)
            nc.vector.tensor_scalar_max(ht[:, h2:], ps1[:, h2:], 0.0)
            state[ss] = (ct, ht)

        def emit_l2(ss):
            bi, s = subs[ss]
            ct, ht = state.pop(ss)
            yt = yts[bi]
            ps2 = pspool.tile([128, sf], F32, name="ps2")
            if sim_mode:
                nc.vector.memset(ps2[:, :], 0.0)
            for q in range(gs):
                qq = s * gs + q
                for j2 in range(2):
                    nc.tensor.matmul(
                        bass.AP(ps2.tensor, 64 * j2 * sf + q * N, [[sf, 2 * D], [1, N]]),
                        bass.AP(ct.tensor, 64 * j2 * cfp + qq * CC + N + KP, [[cfp, KP], [1, 2 * D]]),
                        bass.AP(ht.tensor, 64 * j2 * sf + q * N, [[sf, KP], [1, N]]),
                        start=True,
                        stop=True,
                        tile_position=(64 * j2, 64 * j2),
                    )
            # relu2 split across DVE+ACT halves
            h2 = sf // 2
            nc.vector.tensor_scalar_max(
                bass.AP(yt.tensor, s * sf, [[xfp, 128], [1, h2]]),
                ps2[:, :h2],
                0.0,
            )
            nc.scalar.activation(
                bass.AP(yt.tensor, s * sf + h2, [[xfp, 128], [1, sf - h2]]),
                ps2[:, h2:],
                relu,
            )
            if s == nsub - 1:
                for j2 in range(2):
                    sp_chain(nc.sync.dma_start(
                        bass.AP(yh, (bi * 2 + j2) * 2 * D * xf, [[xf, 2 * D], [1, xf]]),
                        bass.AP(yt.tensor, 64 * j2 * xfp, [[xfp, 2 * D], [1, xf]]),
                    ))
                cts.pop(bi)

        # skew 2: L2(s) is emitted after L1(s+2) so two sub-batches of
        # layer-1 matmuls cover the relu1 chain latency on the in-order PE queue
        SKEW = 1
        for idx in range(len(subs) + SKEW):
            if idx < len(subs):
                emit_l1(idx)
            if idx >= SKEW:
                emit_l2(idx - SKEW)

    _strip_covered_waits(nc)
    _split_excess_waits(nc)
    return nc


def pack_inputs(user_emb, item_emb, nq=QUADS, g=G, dt=np.float32):
    """Shard + lay out inputs for the 8 cores (pair layout)."""
    ncores = NCORES
    nbatch = nq // g
    x = np.ascontiguousarray(user_emb, dtype=np.float32)
    ie = np.ascontiguousarray(item_emb, dtype=np.float32)
    btot = ncores * nq * 4

    comb = np.zeros((btot, K, 83), dtype=np.float32)
    comb[:, :D, :N] = x[:btot].transpose(0, 2, 1)
    comb[:, D, :N] = 1.0
    w = comb[:, :, N:]
    w[:, :D, :D] = ie[:btot, : D * D].reshape(btot, D, D)          # W0
    w[:, D, :D] = ie[:btot, D * D : D * D + D]                     # b0
    w[:, D, D] = 1.0                                               # ones col
    off = D * (D + 1)
    w[:, :D, K : K + D] = ie[:btot, off : off + D * D].reshape(btot, D, D)  # W1
    w[:, D, K : K + D] = ie[:btot, off + D * D : off + D * D + D]  # b1

    # Pair consecutive samples: block-diagonal L1 [34x34], stacked L2 [34x32]
    a, b = comb[0::2], comb[1::2]
    npair = btot // 2
    pc = np.zeros((npair, KP, CC), dtype=np.float32)
    pc[:, :K, :N] = a[:, :, :N]
    pc[:, K:, :N] = b[:, :, :N]
    pc[:, :K, N : N + K] = a[:, :, N : N + K]              # Wt0a
    pc[:, K:, N + K : N + KP] = b[:, :, N : N + K]         # Wt0b
    pc[:, :K, N + KP : N + KP + D] = a[:, :, N + K :]      # Wt1a
    pc[:, K:, N + KP + D : CC] = b[:, :, N + K :]          # Wt1b

    chs = (
        pc.reshape(ncores, nbatch, g, 2, KP, CC)
        .transpose(0, 1, 3, 4, 2, 5)        # c, bi, j2, row, qq, col
        .astype(dt, copy=False)
    )
    return [
        {"ch": np.ascontiguousarray(chs[c]).reshape(nbatch, 2 * KP, g * CC)}
        for c in range(ncores)
    ]


def unpack_output(results, nq=QUADS, g=G):
    """results: per-core {"yh": [nbatch, 64, g*N]} -> full (B, N, D) f32."""
    nbatch = nq // g
    yh = np.stack([r["yh"] for r in results])
    y = (
        yh.reshape(NCORES, nbatch, 2, 2, D, g, N)   # c, bi, j2, h, e, qq, n
        .transpose(0, 1, 5, 2, 3, 6, 4)             # c, bi, qq, j2, h, n, e
    )
    return np.ascontiguousarray(y.reshape(NCORES * nq * 4, N, D))


_NC_CACHE = {}


def _get_nc(key=(QUADS, G)):
    if key not in _NC_CACHE:
        nq, g = key
        _NC_CACHE[key] = build_nc(nq=nq, g=g)
    return _NC_CACHE[key]


def kernel(user_emb, item_emb):
    nc = _get_nc()
    in_maps = pack_inputs(user_emb, item_emb)
    res = run_bass_kernel_spmd(nc, in_maps, core_ids=list(range(NCORES)))
    return unpack_output(res.results)


# revision 41
# speedup vs baseline: 1.5062x; 1.0142x over previous
"""CAN per-sample 2-layer MLP kernel for Trainium2 (8 NeuronCores, SPMD).

Computation (per sample b):
    x = user_emb[b]                           # (50, 16)
    W0, b0, W1, b1 unpacked from item_emb[b]  # (16,16),(16,),(16,16),(16,)
    y = relu(relu(x @ W0 + b0) @ W1 + b1)     # (50, 16)

Mapping:
  * Pure data parallel over 8 cores (2048 samples each).
  * Host packs x^T per sample with an appended ones-row (homogeneous
    coordinates); bias is folded into a 17x17 Wt0 = [[W0,0],[b0,1]] and a
    17x16 Wt1 = [[W1],[b1]] so `x_t @ Wt` applies bias, and the ones row
    self-propagates through layer 1 (relu(1)=1).
  * Samples are PAIRED: each pair is one K=34 matmul (block-diagonal
    34x34 Wt0, stacked 34x32 Wt1), two pairs per 128-partition tile at
    64-row strides running concurrently at PE tile_positions (64j2, 64j2).
    This halves the matmul count vs one matmul per sample for ~25% more
    weight bytes (the off-diagonal zeros).
  * x and W interleave in ONE combined DRAM tensor, batch-major so every
    DMA has a contiguous DRAM side. DMA batches are large (G quads) to
    amortize the ~2us per-lane completion latency; PSUM works in GS-quad
    sub-batches (one bank per tile). Input DMAs are issued 2 batches ahead
    via SWDGE (HWDGE pins DRAM-sourced loads to one SDMA engine) and the
    PE stream is software-pipelined with a one-sub-batch skew; relus are
    split across the ACT and DVE engines.
  * Walrus codegen caps inline sync waits (DMACopy/Matmult: 1): a post-pass
    moves excess waits onto NoOps inserted before the instruction on the
    same queue. SBUF-side DMA APs must stay 2D ([row, nparts], [1, run]) -
    the DIRECT2D DMA struct cannot roll a free run across partitions.
"""

from contextlib import ExitStack

import numpy as np

import concourse.bass as bass
import concourse.mybir as mybir
from concourse import tile
from concourse.bass_utils import run_bass_kernel_spmd
from concourse.tile_rust import add_dep_helper

# Problem constants (hardcoded per contract)
B, N, D = 16384, 50, 16
NCORES = 8
BC = B // NCORES            # 2048 samples per core
K = D + 1                   # 17 rows per sample: 16 features + ones row
KP = 2 * K                  # 34 rows per sample-pair
WP = KP + 2 * D             # 66 weight cols per pair: 34 (L1 blockdiag) + 32 (L2)
CC = N + WP                 # 116 combined cols per pair-slot: x (50) then w (66)
QUADS = BC // 4             # 512 quads of 4 samples per core
G = 32                      # quads per DMA batch
GS = 8                      # quads per PSUM sub-batch (one bank)

F32 = mybir.dt.float32


def _strip_covered_waits(nc):
    """Remove, from DMACopy instructions, semaphore waits already guaranteed
    by an earlier instruction on the same engine queue. Coverage is killed
    for a sem from the point of any non-increment update (barrier resets)."""
    for fn in nc.m.functions:
        for blk in fn.blocks:
            seen = {}
            for ins in blk.instructions:
                si = ins.sync_info
                if si is None:
                    continue
                eng = ins.engine
                strippable = type(ins).__name__ == "InstDMACopy"
                kept = []
                changed = False
                for w in si.on_wait:
                    if (
                        strippable
                        and w.wait_mode == "sem-ge-imm"
                        and w.wait_reg is None
                        and seen.get((eng, w.id), -1) >= w.wait_value
                    ):
                        changed = True
                        continue
                    kept.append(w)
                for w in kept:
                    if w.wait_mode == "sem-ge-imm" and w.wait_reg is None:
                        key = (eng, w.id)
                        if seen.get(key, -1) < w.wait_value:
                            seen[key] = w.wait_value
                for u in si.on_update:
                    if u.update_mode != "sem-add-imm" or (
                        u.update_value is not None and u.update_value < 0
                    ):
                        for key in [k for k in seen if k[1] == u.id]:
                            del seen[key]
                if changed:
                    ins.sync_info = mybir.SyncInfo(
                        on_wait=kept, on_update=si.on_update
                    )


_WS_COUNT = [0]


def _split_excess_waits(nc, cap=1):
    """Move excess inline waits onto NoOps inserted immediately before, on
    the same engine queue - semantically identical (sequencers execute
    waits in order)."""
    for fn in nc.m.functions:
        for blk in fn.blocks:
            insts = blk.instructions
            i = 0
            while i < len(insts):
                ins = insts[i]
                si = ins.sync_info
                if si is None or len(si.on_wait) <= cap:
                    i += 1
                    continue
                waits = list(si.on_wait)
                keep, extra = waits[-cap:], waits[:-cap]
                ins.sync_info = mybir.SyncInfo(on_wait=keep, on_update=si.on_update)
                for w in extra:
                    _WS_COUNT[0] += 1
                    nop = mybir.InstNoOp(name=f"I-ws{_WS_COUNT[0]}", ins=[], outs=[])
                    nop.engine = ins.engine
                    nop.sync_info = mybir.SyncInfo(on_wait=[w], on_update=[])
                    insts.insert(i, nop)
                    i += 1
                i += 1


def build_nc(nq=QUADS, g=G, gs=GS, dt=F32, sim_mode=False):
    """Build the per-core Bass program.

    DRAM (per core), batch-major:
      ch [nbatch, 2*KP, g*CC] : row KP*j2+r, col qq*CC+c = pair (bi*g+qq)*2+j2
      yh [nbatch, 2*2*D, g*N] : row 2*D*j2+e2, col qq*N+n = pair outputs
    """
    assert nq % g == 0 and g % gs == 0
    nbatch = nq // g
    nsub = g // gs
    cf = CC * g              # ct data cols per batch
    xf = N * g               # yt data cols per batch
    sf = N * gs              # psum/ht cols per sub-batch
    cfp = cf + 8             # padded row widths: keep SBUF DMA APs 2D
    xfp = xf + 8

    nc = bass.Bass(
        "TRN2",
        target_bir_lowering=False,
        debug=False,
        detect_race_conditions=False,  # post-pass NoOps confuse its bookkeeping
    )
    ch = nc.dram_tensor("ch", [nbatch, 2 * KP, cf], dt, kind="ExternalInput")
    yh = nc.dram_tensor("yh", [nbatch, 4 * D, xf], F32, kind="ExternalOutput")

    relu = mybir.ActivationFunctionType.Relu

    with tile.TileContext(nc) as tc, ExitStack() as ctx:
        cpool = ctx.enter_context(tc.tile_pool(name="cpool", bufs=3))
        hpool = ctx.enter_context(tc.tile_pool(name="hpool", bufs=3))
        ypool = ctx.enter_context(tc.tile_pool(name="ypool", bufs=2))
        pspool = ctx.enter_context(tc.tile_pool(name="ps", bufs=4, space="PSUM"))

        prev_sp = [None]

        def sp_chain(inst):
            # Pin SP issue order to emission order so prefetched loads are
            # dispatched before later batches' stores.
            if prev_sp[0] is not None:
                add_dep_helper(inst.ins, prev_sp[0].ins, sync=False,
                               reason="SP issue order")
            prev_sp[0] = inst
            return inst

        cts = {}

        def emit_in_dma(bi):
            ct = cpool.tile([128, cfp], dt, name="ct")
            cts[bi] = ct
            # SWDGE: HWDGE pins DRAM-sourced loads to one SDMA engine;
            # SWDGE sprays descriptors across engines by dest partition.
            # The first batch is on the critical path: split its loads
            # column-wise so the earliest sub-batches arrive sooner.
            nsplit = 4 if bi == 0 else 1
            csz = cf // nsplit
            for j2 in range(2):
                for sp in range(nsplit):
                    nc.gpsimd.dma_start(
                        bass.AP(ct.tensor, 64 * j2 * cfp + sp * csz,
                                [[cfp, KP], [1, csz]]),
                        bass.AP(ch, (bi * 2 + j2) * KP * cf + sp * csz,
                                [[cf, KP], [1, csz]]),
                    )

        # 2-deep prefetch prologue
        for pb in range(min(2, nbatch)):
            emit_in_dma(pb)

        # Software-pipelined emission with one-sub-batch skew: the PE queue
        # is in-order, so L2(s) (which waits on relu1(s)) is emitted AFTER
        # L1(s+1) - the PE computes layer 1 of the next sub-batch while
        # relu1(s) runs on ACT, instead of stalling.
        subs = [(bi, s) for bi in range(nbatch) for s in range(nsub)]
        state = {}   # ss -> (ct, yt, ps1, ht)
        yts = {}

        def emit_l1(ss):
            bi, s = subs[ss]
            if s == 0:
                if bi + 2 < nbatch:
                    emit_in_dma(bi + 2)
                yts[bi] = ypool.tile([128, xfp], F32, name="yt")
            ct = cts[bi]
            ps1 = pspool.tile([128, sf], F32, name="ps1")
            if sim_mode:
                nc.vector.memset(ps1[:, :], 0.0)
            for q in range(gs):
                qq = s * gs + q
                for j2 in range(2):
                    nc.tensor.matmul(
                        bass.AP(ps1.tensor, 64 * j2 * sf + q * N, [[sf, KP], [1, N]]),
                        bass.AP(ct.tensor, 64 * j2 * cfp + qq * CC + N, [[cfp, KP], [1, KP]]),
                        bass.AP(ct.tensor, 64 * j2 * cfp + qq * CC, [[cfp, KP], [1, N]]),
                        start=True,
                        stop=True,
                        tile_position=(64 * j2, 64 * j2),
                    )
            ht = hpool.tile([128, sf], dt, name="ht")
            # split relu1 across ACT+DVE halves: halves the latency on the
            # layer-2 critical path; subtile deps let each half's L2 matmuls
            # start independently
            h2 = sf // 2
            nc.scalar.activation(ht[:, :h2], ps1[:, :h2], relu)
            nc.vector.tensor_scalar_max(ht[:, h2:], ps1[:, h2:], 0.0)
            state[ss] = (ct, ht)

        def emit_l2(ss):
            bi, s = subs[ss]
            ct, ht = state.pop(ss)
            yt = yts[bi]
            ps2 = pspool.tile([128, sf], F32, name="ps2")
            if sim_mode:
                nc.vector.memset(ps2[:, :], 0.0)
            for q in range(gs):
                qq = s * gs + q
                for j2 in range(2):
                    nc.tensor.matmul(
                        bass.AP(ps2.tensor, 64 * j2 * sf + q * N, [[sf, 2 * D], [1, N]]),
                        bass.AP(ct.tensor, 64 * j2 * cfp + qq * CC + N + KP, [[cfp, KP], [1, 2 * D]]),
                        bass.AP(ht.tensor, 64 * j2 * sf + q * N, [[sf, KP], [1, N]]),
                        start=True,
                        stop=True,
                        tile_position=(64 * j2, 64 * j2),
                    )
            # relu2 split across DVE+ACT halves
            h2 = sf // 2
            nc.vector.tensor_scalar_max(
                bass.AP(yt.tensor, s * sf, [[xfp, 128], [1, h2]]),
                ps2[:, :h2],
                0.0,
            )
            nc.scalar.activation(
                bass.AP(yt.tensor, s * sf + h2, [[xfp, 128], [1, sf - h2]]),
                ps2[:, h2:],
                relu,
            )
            if s == nsub - 1:
                for j2 in range(2):
                    sp_chain(nc.sync.dma_start(
                        bass.AP(yh, (bi * 2 + j2) * 2 * D * xf, [[xf, 2 * D], [1, xf]]),
                        bass.AP(yt.tensor, 64 * j2 * xfp, [[xfp, 2 * D], [1, xf]]),
                    ))
                cts.pop(bi)

        # skew 2: L2(s) is emitted after L1(s+2) so two sub-batches of
        # layer-1 matmuls cover the relu1 chain latency on the in-order PE queue
        SKEW = 1
        for idx in range(len(subs) + SKEW):
            if idx < len(subs):
                emit_l1(idx)
            if idx >= SKEW:
                emit_l2(idx - SKEW)

    _strip_covered_waits(nc)
    _split_excess_waits(nc)
    return nc


def pack_inputs(user_emb, item_emb, nq=QUADS, g=G, dt=np.float32):
    """Shard + lay out inputs for the 8 cores (pair layout)."""
    ncores = NCORES
    nbatch = nq // g
    x = np.ascontiguousarray(user_emb, dtype=np.float32)
    ie = np.ascontiguousarray(item_emb, dtype=np.float32)
    btot = ncores * nq * 4

    comb = np.zeros((btot, K, 83), dtype=np.float32)
    comb[:, :D, :N] = x[:btot].transpose(0, 2, 1)
    comb[:, D, :N] = 1.0
    w = comb[:, :, N:]
    w[:, :D, :D] = ie[:btot, : D * D].reshape(btot, D, D)          # W0
    w[:, D, :D] = ie[:btot, D * D : D * D + D]                     # b0
    w[:, D, D] = 1.0                                               # ones col
    off = D * (D + 1)
    w[:, :D, K : K + D] = ie[:btot, off : off + D * D].reshape(btot, D, D)  # W1
    w[:, D, K : K + D] = ie[:btot, off + D * D : off + D * D + D]  # b1

    # Pair consecutive samples: block-diagonal L1 [34x34], stacked L2 [34x32]
    a, b = comb[0::2], comb[1::2]
    npair = btot // 2
    pc = np.zeros((npair, KP, CC), dtype=np.float32)
    pc[:, :K, :N] = a[:, :, :N]
    pc[:, K:, :N] = b[:, :, :N]
    pc[:, :K, N : N + K] = a[:, :, N : N + K]              # Wt0a
    pc[:, K:, N + K : N + KP] = b[:, :, N : N + K]         # Wt0b
    pc[:, :K, N + KP : N + KP + D] = a[:, :, N + K :]      # Wt1a
    pc[:, K:, N + KP + D : CC] = b[:, :, N + K :]          # Wt1b

    chs = (
        pc.reshape(ncores, nbatch, g, 2, KP, CC)
        .transpose(0, 1, 3, 4, 2, 5)        # c, bi, j2, row, qq, col
        .astype(dt, copy=False)
    )
    return [
        {"ch": np.ascontiguousarray(chs[c]).reshape(nbatch, 2 * KP, g * CC)}
        for c in range(ncores)
    ]


def unpack_output(results, nq=QUADS, g=G):
    """results: per-core {"yh": [nbatch, 64, g*N]} -> full (B, N, D) f32."""
    nbatch = nq // g
    yh = np.stack([r["yh"] for r in results])
    y = (
        yh.reshape(NCORES, nbatch, 2, 2, D, g, N)   # c, bi, j2, h, e, qq, n
        .transpose(0, 1, 5, 2, 3, 6, 4)             # c, bi, qq, j2, h, n, e
    )
    return np.ascontiguousarray(y.reshape(NCORES * nq * 4, N, D))


_NC_CACHE = {}


def _get_nc(key=(QUADS, G)):
    if key not in _NC_CACHE:
        nq, g = key
        _NC_CACHE[key] = build_nc(nq=nq, g=g)
    return _NC_CACHE[key]


def kernel(user_emb, item_emb):
    nc = _get_nc()
    in_maps = pack_inputs(user_emb, item_emb)
    res = run_bass_kernel_spmd(nc, in_maps, core_ids=list(range(NCORES)))
    return unpack_output(res.results)
